# revision 8
# baseline (speedup 1.0000x reference)
"""Fused rotate(80deg, NN, zero-fill) + roll(7,-13 on H,W) + flip(W,D) for
x[2,128,128,128,8] f32, run as a pixel-gather kernel on 8 NeuronCores.

The whole op is a compile-time-constant gather at pixel granularity: every
output pixel (b,h,w) pulls one 4KB input pixel block (d,f) (or zeros), with
the d-axis reversed inside the block.  Per core (32 output rows):
  - dma_gather (SWDGE): 512 indices x 4KB pixel blocks HBM -> SBUF
  - DVE tensor_copy with a negative-stride AP: reverse d inside each block
  - contiguous 512KB row stores SBUF -> HBM (HWDGE)
Invalid (zero-fill) pixels gather from a zeroed, never-read pixel page.
"""

import numpy as np

_B, _H, _W, _D, _F = 2, 128, 128, 128, 8
_NCORES = 8
_ROWS_PER_CORE = (_B * _H) // _NCORES        # 32
_ROWS_PER_GATHER = 4
_NGATHERS = _ROWS_PER_CORE // _ROWS_PER_GATHER  # 8
_IDXS_PER_GATHER = _ROWS_PER_GATHER * _W     # 512
_IDX_COLS = _IDXS_PER_GATHER // 16           # 32
_PIX = _B * _H * _W                          # 32768 (fits int16 exactly)
_ELEM = _D * _F                              # 1024 f32 = 4096 B
_ROT_DEG = 80.0
_SHIFT_H, _SHIFT_W = 7, -13

_cache = {}


def _index_maps():
    """Mirror reference._rotate_hw's index math exactly (same jax backend as
    the grader) -> (iyc, ixc, valid) as numpy [H, W]."""
    if "maps" in _cache:
        return _cache["maps"]
    import jax.numpy as jnp

    H, W = _H, _W
    theta = jnp.deg2rad(jnp.float32(_ROT_DEG))
    cy, cx = (H - 1) * 0.5, (W - 1) * 0.5
    yy, xx = jnp.meshgrid(
        jnp.arange(H, dtype=jnp.float32),
        jnp.arange(W, dtype=jnp.float32),
        indexing="ij",
    )
    dy, dx = yy - cy, xx - cx
    c, s = jnp.cos(theta), jnp.sin(theta)
    sx = c * dx - s * dy + cx
    sy = s * dx + c * dy + cy
    iy = jnp.round(sy).astype(jnp.int32)
    ix = jnp.round(sx).astype(jnp.int32)
    valid = (iy >= 0) & (iy < H) & (ix >= 0) & (ix < W)
    iyc = jnp.clip(iy, 0, H - 1)
    ixc = jnp.clip(ix, 0, W - 1)
    maps = (np.asarray(iyc), np.asarray(ixc), np.asarray(valid))
    _cache["maps"] = maps
    return maps


def _zero_pixel():
    """A within-batch pixel index never read by any valid gather (verified:
    source pixel (0,0) lies outside the inverse-rotated grid)."""
    iyc, ixc, valid = _index_maps()
    read = np.zeros(_H * _W, dtype=bool)
    read[(iyc[valid] * _W + ixc[valid]).ravel()] = True
    zp = int(np.flatnonzero(~read)[0])
    return zp


def _core_indices():
    """Per-core int16 gather-index arrays [128, _NGATHERS*_IDX_COLS].

    Output pixel (b,h,w) maps through flip/roll to source-grid entry
    (i,j) = ((h-7)%128, (140-w)%128); its input pixel is
    b*16384 + iyc[i,j]*128 + ixc[i,j] when valid[i,j], else the zero page.
    Gather position p (0..511) = (local_row p//128, w p%128) and lives at
    idx[p%16, p//16] of its gather's 32-column block.
    """
    if "idx" in _cache:
        return _cache["idx"]
    iyc, ixc, valid = _index_maps()
    zp = _zero_pixel()
    I = (np.arange(_H) - _SHIFT_H) % _H          # h -> i
    J = (_H + 12 - np.arange(_W)) % _W           # w -> j = (140-w)%128
    V = valid[np.ix_(I, J)]
    LIN = iyc[np.ix_(I, J)].astype(np.int32) * _W + ixc[np.ix_(I, J)]
    per_b = [np.where(V, b * (_H * _W) + LIN, zp).astype(np.int32) for b in range(_B)]
    lin_all = np.concatenate(per_b, axis=0)       # [256 global rows, 128 w]
    assert lin_all.min() >= 0 and lin_all.max() < _PIX

    pos = np.arange(_IDXS_PER_GATHER)
    part, col = pos % 16, pos // 16
    idx_arrays = []
    for k in range(_NCORES):
        arr = np.zeros((128, _NGATHERS * _IDX_COLS), dtype=np.int16)
        for gi in range(_NGATHERS):
            g0 = k * _ROWS_PER_CORE + gi * _ROWS_PER_GATHER
            flat = lin_all[g0 : g0 + _ROWS_PER_GATHER].reshape(-1)  # [512]
            # the 16-partition index block is read per GpSimd Q7 core:
            # replicate it into all 8 16-partition groups
            for c in range(8):
                arr[part + 16 * c, gi * _IDX_COLS + col] = flat.astype(np.int16)
        idx_arrays.append(arr)
    _cache["idx"] = idx_arrays
    return idx_arrays


def _build_nc():
    if "nc" in _cache:
        return _cache["nc"]
    import concourse.bacc as bacc
    import concourse.mybir as mybir
    from concourse.tile import TileContext

    nc = bacc.Bacc("TRN2")
    x = nc.dram_tensor("x", [_PIX, _ELEM], mybir.dt.float32, kind="ExternalInput")
    idx = nc.dram_tensor(
        "idx", [128, _NGATHERS * _IDX_COLS], mybir.dt.int16, kind="ExternalInput"
    )
    # y layout: [w(partition), row, d*f] — one contiguous-partition store per
    # gather group; host assembly transposes (w, row) back to (row, w).
    y = nc.dram_tensor(
        "y", [_W, _ROWS_PER_CORE, _ELEM], mybir.dt.float32, kind="ExternalOutput"
    )
    with TileContext(nc) as tc:
        with (
            tc.tile_pool(name="idxp", bufs=1) as ipool,
            tc.tile_pool(name="warm", bufs=1) as wpool,
            tc.tile_pool(name="gin", bufs=4) as gpool,
            tc.tile_pool(name="gout", bufs=8) as opool,
        ):
            # Warmup gather: forces the Q7 mlp-library load at kernel start,
            # overlapping the preamble + idx DMA instead of stalling gather 0.
            widx = wpool.tile([128, 1], mybir.dt.int16)
            wdst = wpool.tile([128, 1, 64], mybir.dt.float32)
            nc.gpsimd.memset(widx[:, :], 0)
            nc.gpsimd.dma_gather(
                out_ap=wdst[:, :, :],
                in_ap=x[:, 0:64],
                idxs_ap=widx[:, :],
                num_idxs=16,
                num_idxs_reg=16,
                elem_size=64,
                elem_step=_ELEM,
            )
            idxt = ipool.tile([128, _NGATHERS * _IDX_COLS], mybir.dt.int16)
            nc.sync.dma_start(out=idxt[:, :], in_=idx[:, :])
            for gi in range(_NGATHERS):
                tin = gpool.tile(
                    [128, _ROWS_PER_GATHER, _ELEM], mybir.dt.float32, tag="tin"
                )
                nc.gpsimd.dma_gather(
                    out_ap=tin[:, :, :],
                    in_ap=x[:, :],
                    idxs_ap=idxt[:, gi * _IDX_COLS : (gi + 1) * _IDX_COLS],
                    num_idxs=_IDXS_PER_GATHER,
                    num_idxs_reg=_IDXS_PER_GATHER,
                    elem_size=_ELEM,
                )
                rin = tin[:, :, :].rearrange("p r (d f) -> p r d f", f=_F)
                # half-group copies + stores: shorter tail chain, finer overlap
                half = _ROWS_PER_GATHER // 2
                for hh in range(2):
                    tout = opool.tile(
                        [128, half, _D, _F], mybir.dt.float32, tag="tout"
                    )
                    r0 = hh * half
                    nc.vector.tensor_copy(
                        out=tout[:, :, :, :], in_=rin[:, r0 : r0 + half, ::-1, :]
                    )
                    row0 = gi * _ROWS_PER_GATHER + r0
                    nc.sync.dma_start(
                        out=y[:, row0 : row0 + half, :],
                        in_=tout[:, :, :, :].rearrange("p r d f -> p r (d f)"),
                    )
    nc.compile()
    _cache["nc"] = nc
    return nc


def _run(x_np, trace=False):
    from concourse.bass_utils import run_bass_kernel_spmd

    x_arr = np.ascontiguousarray(x_np, dtype=np.float32).reshape(_PIX, _ELEM)
    zp = _zero_pixel()
    xg = x_arr.copy()
    xg[zp] = 0.0
    xg[_H * _W + zp] = 0.0  # batch-1 copy of the zero page (also never read)
    idx_arrays = _core_indices()
    nc = _build_nc()
    in_maps = [{"x": xg, "idx": idx_arrays[k]} for k in range(_NCORES)]
    try:
        res = run_bass_kernel_spmd(
            nc, in_maps, core_ids=list(range(_NCORES)), trace=trace
        )
    except ModuleNotFoundError:
        res = run_bass_kernel_spmd(
            nc, in_maps, core_ids=list(range(_NCORES)), trace=False
        )
    out = np.empty((_B * _H, _W, _ELEM), dtype=np.float32)
    for k in range(_NCORES):
        yk = res.results[k]["y"]  # [w, row, elem]
        out[k * _ROWS_PER_CORE : (k + 1) * _ROWS_PER_CORE] = yk.transpose(1, 0, 2)
    out = out.reshape(_B, _H, _W, _D, _F)
    return out, res


def kernel(x):
    out, _ = _run(np.asarray(x), trace=False)
    return out


def run_traced(x):
    """Like kernel() but with NTFF profiling; returns (out, BassKernelResults)."""
    return _run(np.asarray(x), trace=True)


# revision 10
# speedup vs baseline: 1.0370x; 1.0370x over previous
"""Fused rotate(80deg, NN, zero-fill) + roll(7,-13 on H,W) + flip(W,D) for
x[2,128,128,128,8] f32, run as a pixel-gather kernel on 8 NeuronCores.

The whole op is a compile-time-constant gather at pixel granularity: every
output pixel (b,h,w) pulls one 4KB input pixel block (d,f) (or zeros), with
the d-axis reversed inside the block.  Per core (32 output rows):
  - dma_gather (SWDGE): 512 indices x 4KB pixel blocks HBM -> SBUF
  - DVE tensor_copy with a negative-stride AP: reverse d inside each block
  - contiguous 512KB row stores SBUF -> HBM (HWDGE)
Invalid (zero-fill) pixels gather from a zeroed, never-read pixel page.
"""

import numpy as np

_B, _H, _W, _D, _F = 2, 128, 128, 128, 8
_NCORES = 8
_ROWS_PER_CORE = (_B * _H) // _NCORES        # 32
_ROWS_PER_GATHER = 4
_NGATHERS = _ROWS_PER_CORE // _ROWS_PER_GATHER  # 8
_IDXS_PER_GATHER = _ROWS_PER_GATHER * _W     # 512
_IDX_COLS = _IDXS_PER_GATHER // 16           # 32
_PIX = _B * _H * _W                          # 32768 (fits int16 exactly)
_ELEM = _D * _F                              # 1024 f32 = 4096 B
_ROT_DEG = 80.0
_SHIFT_H, _SHIFT_W = 7, -13

_cache = {}


def _index_maps():
    """Mirror reference._rotate_hw's index math exactly (same jax backend as
    the grader) -> (iyc, ixc, valid) as numpy [H, W]."""
    if "maps" in _cache:
        return _cache["maps"]
    import jax.numpy as jnp

    H, W = _H, _W
    theta = jnp.deg2rad(jnp.float32(_ROT_DEG))
    cy, cx = (H - 1) * 0.5, (W - 1) * 0.5
    yy, xx = jnp.meshgrid(
        jnp.arange(H, dtype=jnp.float32),
        jnp.arange(W, dtype=jnp.float32),
        indexing="ij",
    )
    dy, dx = yy - cy, xx - cx
    c, s = jnp.cos(theta), jnp.sin(theta)
    sx = c * dx - s * dy + cx
    sy = s * dx + c * dy + cy
    iy = jnp.round(sy).astype(jnp.int32)
    ix = jnp.round(sx).astype(jnp.int32)
    valid = (iy >= 0) & (iy < H) & (ix >= 0) & (ix < W)
    iyc = jnp.clip(iy, 0, H - 1)
    ixc = jnp.clip(ix, 0, W - 1)
    maps = (np.asarray(iyc), np.asarray(ixc), np.asarray(valid))
    _cache["maps"] = maps
    return maps


def _zero_pixel():
    """A within-batch pixel index never read by any valid gather (verified:
    source pixel (0,0) lies outside the inverse-rotated grid)."""
    iyc, ixc, valid = _index_maps()
    read = np.zeros(_H * _W, dtype=bool)
    read[(iyc[valid] * _W + ixc[valid]).ravel()] = True
    zp = int(np.flatnonzero(~read)[0])
    return zp


def _core_indices():
    """Per-core int16 gather-index arrays [128, _NGATHERS*_IDX_COLS].

    Output pixel (b,h,w) maps through flip/roll to source-grid entry
    (i,j) = ((h-7)%128, (140-w)%128); its input pixel is
    b*16384 + iyc[i,j]*128 + ixc[i,j] when valid[i,j], else the zero page.
    Gather position p (0..511) = (local_row p//128, w p%128) and lives at
    idx[p%16, p//16] of its gather's 32-column block.
    """
    if "idx" in _cache:
        return _cache["idx"]
    iyc, ixc, valid = _index_maps()
    zp = _zero_pixel()
    I = (np.arange(_H) - _SHIFT_H) % _H          # h -> i
    J = (_H + 12 - np.arange(_W)) % _W           # w -> j = (140-w)%128
    V = valid[np.ix_(I, J)]
    LIN = iyc[np.ix_(I, J)].astype(np.int32) * _W + ixc[np.ix_(I, J)]
    per_b = [np.where(V, b * (_H * _W) + LIN, zp).astype(np.int32) for b in range(_B)]
    lin_all = np.concatenate(per_b, axis=0)       # [256 global rows, 128 w]
    assert lin_all.min() >= 0 and lin_all.max() < _PIX

    pos = np.arange(_IDXS_PER_GATHER)
    part, col = pos % 16, pos // 16
    idx_arrays = []
    for k in range(_NCORES):
        arr = np.zeros((128, _NGATHERS * _IDX_COLS), dtype=np.int16)
        for gi in range(_NGATHERS):
            g0 = k * _ROWS_PER_CORE + gi * _ROWS_PER_GATHER
            flat = lin_all[g0 : g0 + _ROWS_PER_GATHER].reshape(-1)  # [512]
            # the 16-partition index block is read per GpSimd Q7 core:
            # replicate it into all 8 16-partition groups
            for c in range(8):
                arr[part + 16 * c, gi * _IDX_COLS + col] = flat.astype(np.int16)
        idx_arrays.append(arr)
    _cache["idx"] = idx_arrays
    return idx_arrays


def _build_nc():
    if "nc" in _cache:
        return _cache["nc"]
    import concourse.bacc as bacc
    import concourse.mybir as mybir
    from concourse.tile import TileContext

    nc = bacc.Bacc("TRN2")
    x = nc.dram_tensor("x", [_PIX, _ELEM], mybir.dt.float32, kind="ExternalInput")
    idx = nc.dram_tensor(
        "idx", [128, _NGATHERS * _IDX_COLS], mybir.dt.int16, kind="ExternalInput"
    )
    # y layout: [w(partition), row, d*f] — one contiguous-partition store per
    # gather group; host assembly transposes (w, row) back to (row, w).
    y = nc.dram_tensor(
        "y", [_W, _ROWS_PER_CORE, _ELEM], mybir.dt.float32, kind="ExternalOutput"
    )
    # Load the Q7 library (gather ucode) before the Tile region so the
    # ~10us reload overlaps the kernel preamble instead of stalling gather 0.
    from concourse.library_config import mlp

    nc.gpsimd.load_library(mlp)
    with TileContext(nc) as tc:
        with (
            tc.tile_pool(name="idxp", bufs=1) as ipool,
            tc.tile_pool(name="gin", bufs=4) as gpool,
            tc.tile_pool(name="gout", bufs=4) as opool,
        ):
            idxt = ipool.tile([128, _NGATHERS * _IDX_COLS], mybir.dt.int16)
            nc.sync.dma_start(out=idxt[:, :], in_=idx[:, :])
            for gi in range(_NGATHERS):
                tin = gpool.tile(
                    [128, _ROWS_PER_GATHER, _ELEM], mybir.dt.float32, tag="tin"
                )
                nc.gpsimd.dma_gather(
                    out_ap=tin[:, :, :],
                    in_ap=x[:, :],
                    idxs_ap=idxt[:, gi * _IDX_COLS : (gi + 1) * _IDX_COLS],
                    num_idxs=_IDXS_PER_GATHER,
                    num_idxs_reg=_IDXS_PER_GATHER,
                    elem_size=_ELEM,
                )
                rin = tin[:, :, :].rearrange("p r (d f) -> p r d f", f=_F)
                # 2-row DVE copies (hit the fast path: 0.6 vs 1.35 ns/elem)
                # into one 4-row tile, stored as a single 2MB DMA (16KB
                # packets run at ~27 B/ns vs 20.6 for 8KB).
                tout = opool.tile(
                    [128, _ROWS_PER_GATHER, _D, _F], mybir.dt.float32, tag="tout"
                )
                half = _ROWS_PER_GATHER // 2
                for hh in range(2):
                    r0 = hh * half
                    nc.vector.tensor_copy(
                        out=tout[:, r0 : r0 + half, :, :],
                        in_=rin[:, r0 : r0 + half, ::-1, :],
                    )
                row0 = gi * _ROWS_PER_GATHER
                nc.sync.dma_start(
                    out=y[:, row0 : row0 + _ROWS_PER_GATHER, :],
                    in_=tout[:, :, :, :].rearrange("p r d f -> p r (d f)"),
                )
    nc.compile()
    _cache["nc"] = nc
    return nc


def _run(x_np, trace=False):
    from concourse.bass_utils import run_bass_kernel_spmd

    x_arr = np.ascontiguousarray(x_np, dtype=np.float32).reshape(_PIX, _ELEM)
    zp = _zero_pixel()
    xg = x_arr.copy()
    xg[zp] = 0.0
    xg[_H * _W + zp] = 0.0  # batch-1 copy of the zero page (also never read)
    idx_arrays = _core_indices()
    nc = _build_nc()
    in_maps = [{"x": xg, "idx": idx_arrays[k]} for k in range(_NCORES)]
    try:
        res = run_bass_kernel_spmd(
            nc, in_maps, core_ids=list(range(_NCORES)), trace=trace
        )
    except ModuleNotFoundError:
        res = run_bass_kernel_spmd(
            nc, in_maps, core_ids=list(range(_NCORES)), trace=False
        )
    out = np.empty((_B * _H, _W, _ELEM), dtype=np.float32)
    for k in range(_NCORES):
        yk = res.results[k]["y"]  # [w, row, elem]
        out[k * _ROWS_PER_CORE : (k + 1) * _ROWS_PER_CORE] = yk.transpose(1, 0, 2)
    out = out.reshape(_B, _H, _W, _D, _F)
    return out, res


def kernel(x):
    out, _ = _run(np.asarray(x), trace=False)
    return out


def run_traced(x):
    """Like kernel() but with NTFF profiling; returns (out, BassKernelResults)."""
    return _run(np.asarray(x), trace=True)


# revision 31
# speedup vs baseline: 1.2675x; 1.2223x over previous
"""Fused rotate(80deg, NN, zero-fill) + roll(7,-13 on H,W) + flip(W,D) for
x[2,128,128,128,8] f32, run as a pixel-gather kernel on 8 NeuronCores.

The whole op is a compile-time-constant gather at pixel granularity: every
output pixel (b,h,w) pulls one 4KB input pixel block (d,f) (or zeros), with
the d-axis reversed inside the block.  Per core (32 output rows):
  - dma_gather (SWDGE): 512 indices x 4KB pixel blocks HBM -> SBUF
  - DVE tensor_copy with a negative-stride AP: reverse d inside each block
  - contiguous 512KB row stores SBUF -> HBM (HWDGE)
Invalid (zero-fill) pixels gather from a zeroed, never-read pixel page.
"""

import numpy as np

_B, _H, _W, _D, _F = 2, 128, 128, 128, 8
_NCORES = 8
_ROWS_PER_CORE = (_B * _H) // _NCORES        # 32
_ROWS_PER_GATHER = 4
_NGATHERS = _ROWS_PER_CORE // _ROWS_PER_GATHER  # 8
_IDXS_PER_GATHER = _ROWS_PER_GATHER * _W     # 512
_IDX_COLS = _IDXS_PER_GATHER // 16           # 32
_PIX = _B * _H * _W                          # 32768 (fits int16 exactly)
_ELEM = _D * _F                              # 1024 f32 = 4096 B
_ROT_DEG = 80.0
_SHIFT_H, _SHIFT_W = 7, -13

_cache = {}


def _index_maps():
    """Mirror reference._rotate_hw's index math exactly (same jax backend as
    the grader) -> (iyc, ixc, valid) as numpy [H, W]."""
    if "maps" in _cache:
        return _cache["maps"]
    import jax.numpy as jnp

    H, W = _H, _W
    theta = jnp.deg2rad(jnp.float32(_ROT_DEG))
    cy, cx = (H - 1) * 0.5, (W - 1) * 0.5
    yy, xx = jnp.meshgrid(
        jnp.arange(H, dtype=jnp.float32),
        jnp.arange(W, dtype=jnp.float32),
        indexing="ij",
    )
    dy, dx = yy - cy, xx - cx
    c, s = jnp.cos(theta), jnp.sin(theta)
    sx = c * dx - s * dy + cx
    sy = s * dx + c * dy + cy
    iy = jnp.round(sy).astype(jnp.int32)
    ix = jnp.round(sx).astype(jnp.int32)
    valid = (iy >= 0) & (iy < H) & (ix >= 0) & (ix < W)
    iyc = jnp.clip(iy, 0, H - 1)
    ixc = jnp.clip(ix, 0, W - 1)
    maps = (np.asarray(iyc), np.asarray(ixc), np.asarray(valid))
    _cache["maps"] = maps
    return maps


def _zero_pixel():
    """A within-batch pixel index never read by any valid gather (verified:
    source pixel (0,0) lies outside the inverse-rotated grid)."""
    iyc, ixc, valid = _index_maps()
    read = np.zeros(_H * _W, dtype=bool)
    read[(iyc[valid] * _W + ixc[valid]).ravel()] = True
    zp = int(np.flatnonzero(~read)[0])
    return zp


_OOB = 0x40000000  # > any pixel index; bounds_check skips these (no transfer)


def _core_indices(skip_oob=False):
    """Per-core int32 index arrays [128, 32]: idx[p, r] = source pixel of
    output (row r, w=p), where output pixel (b,h,w) maps through flip/roll
    to source-grid entry (i,j) = ((h-7)%128, (140-w)%128) and pixel
    b*16384 + iyc[i,j]*128 + ixc[i,j] when valid[i,j].  Invalid entries are
    the zero page (skip_oob=False) or _OOB sentinels skipped by the
    bounds-checked gather (skip_oob=True; the tile is pre-zeroed on DVE).
    """
    key = ("idx", skip_oob)
    if key in _cache:
        return _cache[key]
    iyc, ixc, valid = _index_maps()
    inv = _OOB if skip_oob else _zero_pixel()
    I = (np.arange(_H) - _SHIFT_H) % _H          # h -> i
    J = (_H + 12 - np.arange(_W)) % _W           # w -> j = (140-w)%128
    V = valid[np.ix_(I, J)]
    LIN = iyc[np.ix_(I, J)].astype(np.int32) * _W + ixc[np.ix_(I, J)]
    per_b = [np.where(V, b * (_H * _W) + LIN, inv).astype(np.int32) for b in range(_B)]
    lin_all = np.concatenate(per_b, axis=0)       # [256 global rows (h), 128 w]

    idx_arrays = []
    for k in range(_NCORES):
        rows = lin_all[k * _ROWS_PER_CORE : (k + 1) * _ROWS_PER_CORE]  # [32,128]
        idx_arrays.append(np.ascontiguousarray(rows.T).astype(np.int32))  # [128,32]
    _cache[key] = idx_arrays
    return idx_arrays


def _build_nc():
    if "nc" in _cache:
        return _cache["nc"]
    import os

    import concourse.bacc as bacc
    import concourse.mybir as mybir
    from concourse.tile import TileContext

    from contextlib import ExitStack

    import concourse.bass as bass

    gin_bufs = int(os.environ.get("KV_GIN", "4"))
    gout_bufs = int(os.environ.get("KV_GOUT", "4"))
    early_idx = os.environ.get("KV_EARLY_IDX", "0") == "1"
    use_skip = os.environ.get("KV_SKIP_OOB", "0") == "1"
    taper = os.environ.get("KV_TAPER", "1") == "1"

    nc = bacc.Bacc("TRN2")
    x = nc.dram_tensor("x", [_PIX, _ELEM], mybir.dt.float32, kind="ExternalInput")
    idx = nc.dram_tensor(
        "idx", [128, _ROWS_PER_CORE], mybir.dt.int32, kind="ExternalInput"
    )
    # y layout: [w(partition), row, d*f] — one contiguous-partition store per
    # gather group; host assembly transposes (w, row) back to (row, w).
    y = nc.dram_tensor(
        "y", [_W, _ROWS_PER_CORE, _ELEM], mybir.dt.float32, kind="ExternalOutput"
    )

    es = ExitStack()
    _cache["es"] = es
    idxt = None
    idx_sem = None
    if early_idx:
        idxt = es.enter_context(
            nc.sbuf_tensor("idxt_pre", [128, _ROWS_PER_CORE], mybir.dt.int32)
        )
        idx_sem = es.enter_context(nc.semaphore("idx_sem"))
        nc.sync.dma_start(out=idxt[:, :], in_=idx[:, :]).then_inc(idx_sem, 16)

    if taper:
        group_sizes = [2, 2] + [4] * 6 + [2, 2]
    else:
        group_sizes = [_ROWS_PER_GATHER] * _NGATHERS

    with TileContext(nc) as tc:
        with (
            tc.tile_pool(name="idxp", bufs=1) as ipool,
            tc.tile_pool(name="gin", bufs=gin_bufs) as gpool,
            tc.tile_pool(name="gout", bufs=gout_bufs) as opool,
        ):
            if early_idx:
                with tc.tile_critical():
                    nc.gpsimd.wait_ge(idx_sem, 16)
            else:
                idxt = ipool.tile([128, _ROWS_PER_CORE], mybir.dt.int32)
                # tiny first-columns DMA unblocks gather 0 ~1us sooner than
                # waiting for the full 16KB index load
                nc.sync.dma_start(out=idxt[:, 0:2], in_=idx[:, 0:2])
                nc.sync.dma_start(out=idxt[:, 2:], in_=idx[:, 2:])
            row0 = 0
            for gsize in group_sizes:
                tin = gpool.tile([128, gsize, _ELEM], mybir.dt.float32, tag="tin")
                if use_skip:
                    # pre-zero the tile; bounds-checked gathers skip invalid
                    # pixels (no HBM read) and leave the zeros in place
                    nc.vector.memset(tin[:, :, :], 0.0)
                for r_ in range(gsize):
                    row = row0 + r_
                    if use_skip:
                        nc.gpsimd.indirect_dma_start(
                            out=tin[:, r_, :],
                            out_offset=None,
                            in_=x[:, :],
                            in_offset=bass.IndirectOffsetOnAxis(
                                ap=idxt[:, row : row + 1], axis=0
                            ),
                            bounds_check=_PIX - 1,
                            oob_is_err=False,
                        )
                    else:
                        nc.gpsimd.indirect_dma_start(
                            out=tin[:, r_, :],
                            out_offset=None,
                            in_=x[:, :],
                            in_offset=bass.IndirectOffsetOnAxis(
                                ap=idxt[:, row : row + 1], axis=0
                            ),
                        )
                rin = tin[:, :, :].rearrange("p r (d f) -> p r d f", f=_F)
                tout = opool.tile([128, gsize, _D, _F], mybir.dt.float32, tag="tout")
                # 2-row reversed-d copies hit the DVE fast path; one 2MB
                # store per group keeps 16KB DMA packets (~27 B/ns)
                for hh in range(0, gsize, 2):
                    nc.vector.tensor_copy(
                        out=tout[:, hh : hh + 2, :, :],
                        in_=rin[:, hh : hh + 2, ::-1, :],
                    )
                nc.sync.dma_start(
                    out=y[:, row0 : row0 + gsize, :],
                    in_=tout[:, :, :, :].rearrange("p r d f -> p r (d f)"),
                )
                row0 += gsize
    nc.compile()
    _cache["nc"] = nc
    return nc


def _run(x_np, trace=False):
    from concourse.bass_utils import run_bass_kernel_spmd

    import os

    x_arr = np.ascontiguousarray(x_np, dtype=np.float32).reshape(_PIX, _ELEM)
    use_skip = os.environ.get("KV_SKIP_OOB", "0") == "1"
    if use_skip:
        xg = x_arr
    else:
        xg = x_arr.copy()
        xg[_zero_pixel()] = 0.0
    idx_arrays = _core_indices(skip_oob=use_skip)
    nc = _build_nc()
    in_maps = [{"x": xg, "idx": idx_arrays[k]} for k in range(_NCORES)]
    kwargs = {}
    if trace:
        import os

        if os.environ.get("KV_ALLCORES") == "1":
            kwargs["trace_cores"] = list(range(_NCORES))
    try:
        res = run_bass_kernel_spmd(
            nc, in_maps, core_ids=list(range(_NCORES)), trace=trace, **kwargs
        )
    except ModuleNotFoundError:
        res = run_bass_kernel_spmd(
            nc, in_maps, core_ids=list(range(_NCORES)), trace=False
        )
    out = np.empty((_B * _H, _W, _ELEM), dtype=np.float32)
    for k in range(_NCORES):
        yk = res.results[k]["y"]  # [w, row, elem]
        out[k * _ROWS_PER_CORE : (k + 1) * _ROWS_PER_CORE] = yk.transpose(1, 0, 2)
    out = out.reshape(_B, _H, _W, _D, _F)
    return out, res


def kernel(x):
    out, _ = _run(np.asarray(x), trace=False)
    return out


def run_traced(x):
    """Like kernel() but with NTFF profiling; returns (out, BassKernelResults)."""
    return _run(np.asarray(x), trace=True)
